# revision 35
# baseline (speedup 1.0000x reference)
"""Trainium2 Bass kernel for nn_MultiHeadAttention (B=4, S=2048, D=1024, H=16).

Sharding: 8 cores = 4 batches x 2 head-groups. Core c handles batch c//2,
heads [8*(c%2), 8*(c%2)+8). Each core computes qkv for its 8 heads,
attention, and a partial c_proj using its 512 rows of W_proj. Host sums
the two partials per batch (the "all-reduce after c_proj").

v3 (all-bf16 matmul path, f32 PSUM accumulation):
  - whole x^T + all weights resident in SBUF; per-kc tiles so the first
    qkv chains start as soon as their own DMA slices land
  - per pair p: Q^T, K^T [128(=2 heads x 64), S] bf16, built via filler
    chains just-in-time; V^T likewise during the first attention block
  - attention in (p, qc) blocks of 512 q columns; per key block kb one
    scores PSUM tile [128, 2, 512] (h-major), one exp -> pt bf16; pt pairs
    two kb so denominator adds are single [128, 2048] DVE instrs
  - softmax denominator: bf16 A/B chains, folded on DVE, partition-reduced
    on GpSimd (partition_all_reduce) -- no tensor-engine ones-matmuls
  - c_proj per qc chunk as filler once all pairs finish that chunk;
    bf16 partial output, host sums in f32
"""

import contextlib
import ctypes
import os
import sys
import types
from collections import deque

import numpy as np

# ---------------------------------------------------------------------------
# NTFF profiling hook (used when BASS_PROBLEM_TRACE=1): the agent image lacks
# antenv.axon_hooks, so provide it via ctypes against libaxon_pjrt.so.
# ---------------------------------------------------------------------------
_AXON_SO = "/opt/axon/libaxon_pjrt.so"


def _install_ntff_hook():
    if "antenv.axon_hooks" in sys.modules:
        return
    try:
        import antenv
    except ImportError:
        return
    try:
        lib = ctypes.CDLL(_AXON_SO)
    except OSError:
        return
    if not hasattr(lib, "axon_start_nrt_profile"):
        return
    lib.axon_start_nrt_profile.argtypes = [
        ctypes.POINTER(ctypes.c_int64),
        ctypes.c_size_t,
    ]
    lib.axon_start_nrt_profile.restype = ctypes.c_int64
    lib.axon_stop_nrt_profile.argtypes = [ctypes.c_char_p]
    lib.axon_stop_nrt_profile.restype = ctypes.c_int64

    @contextlib.contextmanager
    def _hook(output_dir, device_ids):
        import jax

        jax.devices()
        if device_ids:
            ids = (ctypes.c_int64 * len(device_ids))(*device_ids)
            rc = lib.axon_start_nrt_profile(ids, len(device_ids))
        else:
            rc = lib.axon_start_nrt_profile(None, 0)
        if rc != 0:
            raise RuntimeError(f"axon_start_nrt_profile rc={rc}")
        try:
            yield
        finally:
            n = lib.axon_stop_nrt_profile(str(output_dir).encode())
            print(f"profile: {n} file(s) written to {output_dir}", file=sys.stderr)

    mod = types.ModuleType("antenv.axon_hooks")
    holder = [_hook]
    mod.get_axon_ntff_profile_hook = lambda: holder[0]
    mod.set_axon_ntff_profile_hook = lambda h: holder.__setitem__(0, h)
    sys.modules["antenv.axon_hooks"] = mod
    antenv.axon_hooks = mod


_install_ntff_hook()

# ---------------------------------------------------------------------------
# Problem constants (hardcoded per the contract)
# ---------------------------------------------------------------------------
B, S, D = 4, 2048, 1024
H, DK = 16, 64
N_CORES = 8
HPC = 8           # heads per core
NPAIR = HPC // 2  # head pairs per core = 4
FC = HPC * DK     # features per core = 512
SCALE = 1.0 / float(np.sqrt(DK))  # 0.125

KC = D // 128     # 8 contraction chunks
KB = S // 128     # 16 key blocks
NQC = 4           # q chunks per pair
QCW = S // NQC    # 512

_CACHED = {}


def _build():
    import concourse.tile as tile
    from concourse import bacc, bass_isa, mybir

    f32 = mybir.dt.float32
    bf16 = mybir.dt.bfloat16
    Exp = mybir.ActivationFunctionType.Exp
    RAdd = bass_isa.ReduceOp.add

    nc = bacc.Bacc("TRN2", target_bir_lowering=False, debug=False,
                   num_devices=N_CORES)

    xt = nc.dram_tensor("xt", [D, S], bf16, kind="ExternalInput").ap()
    wq = nc.dram_tensor("wq", [D, FC], bf16, kind="ExternalInput").ap()
    wk = nc.dram_tensor("wk", [D, FC], bf16, kind="ExternalInput").ap()
    wv = nc.dram_tensor("wv", [D, FC], bf16, kind="ExternalInput").ap()
    wp = nc.dram_tensor("wp", [128, NPAIR, D], bf16, kind="ExternalInput").ap()
    out = nc.dram_tensor("out", [S, D], bf16, kind="ExternalOutput").ap()

    with tile.TileContext(nc) as tc:
        with (
            tc.tile_pool(name="res", bufs=1) as res_pool,
            tc.tile_pool(name="qkt", bufs=3) as qkt_pool,
            tc.tile_pool(name="ptp", bufs=6) as pt_pool,
            tc.tile_pool(name="acc", bufs=2) as acc_pool,
            tc.tile_pool(name="dnq", bufs=2) as den_pool,
            tc.tile_pool(name="ibp", bufs=2) as ibc_pool,
            tc.tile_pool(name="rcp", bufs=2) as rec_pool,
            tc.tile_pool(name="osb", bufs=3) as out_pool,
            tc.tile_pool(name="stp", bufs=2, space="PSUM") as st_ps,
            tc.tile_pool(name="avp", bufs=2, space="PSUM") as av_ps,
            tc.tile_pool(name="chn", bufs=1, space="PSUM") as chain_ps,
            tc.tile_pool(name="dsp", bufs=1, space="PSUM") as dps_ps,
        ):
            # ------------- resident SBUF tensors ------------------------
            XT = res_pool.tile([128, KC, S], bf16, tag="XT")
            WQ = res_pool.tile([128, KC, FC], bf16, tag="WQ")
            WK = res_pool.tile([128, KC, FC], bf16, tag="WK")
            WV = res_pool.tile([128, KC, FC], bf16, tag="WV")
            WP = res_pool.tile([128, NPAIR, D], bf16, tag="WP")
            VT = res_pool.tile([128, KB, FC], bf16, tag="VT")
            ATN = res_pool.tile([128, NPAIR, S], bf16, tag="ATN")
            ones = res_pool.tile([128, 1], bf16, tag="ones")
            nc.gpsimd.memset(ones[:], 1.0)
            scr = res_pool.tile([128, 512], bf16, tag="scr")
            nc.gpsimd.memset(scr[:], 0.0)

            # input DMA: one multi-dim descriptor per wave (sync-queue
            # issue time dominates many small DMAs). Wave order is
            # first-needed first: x sc0 + pair-0 cols of WQ/WK, x sc1,
            # WV, remaining W cols, x sc2/3, WP.
            xt3 = xt.rearrange("(kc p) s -> p kc s", kc=KC)
            wq3 = wq.rearrange("(kc p) f -> p kc f", kc=KC)
            wk3 = wk.rearrange("(kc p) f -> p kc f", kc=KC)
            wv3 = wv.rearrange("(kc p) f -> p kc f", kc=KC)

            def dma_xt(csl):
                nc.sync.dma_start(XT[:, :, csl], xt3[:, :, csl])

            def dma_w(dst, src3, csl):
                nc.sync.dma_start(dst[:, :, csl], src3[:, :, csl])

            dma_xt(slice(0, 512))
            dma_w(WQ, wq3, slice(0, 128))
            dma_w(WK, wk3, slice(0, 128))
            dma_xt(slice(512, 1024))
            dma_w(WV, wv3, slice(0, FC))
            dma_xt(slice(1024, 1536))
            dma_w(WQ, wq3, slice(128, FC))
            dma_w(WK, wk3, slice(128, FC))
            dma_xt(slice(1536, 2048))
            nc.sync.dma_start(WP[:, :, :], wp[:, :, :])

            QT = {}
            KT = {}

            # ------------- chain emitters (tensor matmul chains) ----------
            def emit_qk_chain(dst, w, p, sc, nm):
                """One 512-col slice of Q^T or K^T for pair p."""
                ps = chain_ps.tile([128, 512], f32, tag="chain",
                                   name=f"qk_{nm}_{p}_{sc}")
                ssl = slice(sc * 512, (sc + 1) * 512)
                for kc in range(KC):
                    nc.tensor.matmul(ps[:],
                                     lhsT=w[:, kc, p * 128:(p + 1) * 128],
                                     rhs=XT[:, kc, ssl],
                                     start=(kc == 0), stop=(kc == KC - 1))
                nc.vector.tensor_copy(dst[:, ssl], ps[:])

            def build_qk(p):
                QT[p] = qkt_pool.tile([128, S], bf16, tag="QT", name=f"QT{p}")
                KT[p] = qkt_pool.tile([128, S], bf16, tag="KT", name=f"KT{p}")

            def q_chain(p, sc):
                return lambda: emit_qk_chain(QT[p], WQ, p, sc, "q")

            def k_chain(p, sc):
                return lambda: emit_qk_chain(KT[p], WK, p, sc, "k")

            def emit_v_chain(kb):
                """V^T for key block kb: [128 keys, FC]."""
                ps = chain_ps.tile([128, FC], f32, tag="chain", name=f"v_{kb}")
                for kc in range(KC):
                    nc.tensor.matmul(ps[:],
                                     lhsT=XT[:, kc, kb * 128:(kb + 1) * 128],
                                     rhs=WV[:, kc, :],
                                     start=(kc == 0), stop=(kc == KC - 1))
                nc.vector.tensor_copy(VT[:, kb, :], ps[:])

            def emit_cproj_chain(sb, nn, pool=None):
                """c_proj for 128 q rows x 512 out cols."""
                ps = (pool or chain_ps).tile([128, 512], f32,
                                             tag="chain" if pool is None
                                             else "avps",
                                             name=f"cp_{sb}_{nn}")
                for p in range(NPAIR):
                    nc.tensor.matmul(ps[:],
                                     lhsT=ATN[:, p, sb * 128:(sb + 1) * 128],
                                     rhs=WP[:, p, nn * 512:(nn + 1) * 512],
                                     start=(p == 0), stop=(p == NPAIR - 1))
                ot = out_pool.tile([128, 512], bf16, tag="ot",
                                   name=f"ot_{sb}_{nn}")
                nc.vector.tensor_copy(ot[:], ps[:])
                nc.sync.dma_start(
                    out[sb * 128:(sb + 1) * 128, nn * 512:(nn + 1) * 512],
                    ot[:])

            def cproj_fillers(qc):
                return [(lambda sb=sb, nn=nn: emit_cproj_chain(sb, nn))
                        for sb in range(qc * 4, qc * 4 + 4) for nn in range(2)]

            # ------------- boundary: softmax denominator + normalize ------
            def emit_boundary(pend):
                (bp, bqc, bavs, baccA, baccB) = pend
                qsl = slice(bqc * QCW, (bqc + 1) * QCW)
                den = den_pool.tile([128, 2, 2, 512], bf16, tag="den",
                                    name=f"den_{bp}_{bqc}")
                nc.vector.tensor_add(den[:], baccA[:], baccB[:])
                den2 = den_pool.tile([128, 2, 512], bf16, tag="den2",
                                     name=f"den2_{bp}_{bqc}")
                nc.vector.tensor_add(den2[:], den[:, :, 0, :], den[:, :, 1, :])
                inv = rec_pool.tile([1, 2, 512], f32, tag="inv",
                                    name=f"inv_{bp}_{bqc}")
                for h in range(2):
                    dps = dps_ps.tile([1, 512], f32, tag="dps",
                                      name=f"dps_{bp}_{bqc}_{h}")
                    nc.tensor.matmul(dps[:], lhsT=ones[:], rhs=den2[:, h, :],
                                     start=True, stop=True)
                    nc.vector.reciprocal_approx_fast(inv[0:1, h, :], dps[:])
                ib = ibc_pool.tile([128, 2, 512], f32, tag="ibc",
                                   name=f"ibc_{bp}_{bqc}")
                for h in range(2):
                    nc.gpsimd.partition_broadcast(ib[:, h, :], inv[0:1, h, :])
                for h in range(2):
                    hsl = slice(64 * h, 64 * h + 64)
                    nc.vector.tensor_mul(ATN[hsl, bp, qsl], bavs[hsl, :],
                                         ib[hsl, h, :])

            # ------------- prologue -------------------------------------
            # PE clock warmup: dummy matmuls with no DMA deps run during
            # the input-DMA wait, so the real chains start at full clock
            wps = chain_ps.tile([128, 512], f32, tag="chain", name="warm")
            for _ in range(18):
                nc.tensor.matmul(wps[:], lhsT=scr[:, 0:128], rhs=scr[:],
                                 start=True, stop=True)
            nc.vector.tensor_copy(scr[0:1, 0:4], wps[0:1, 0:4])
            build_qk(0)
            q_chain(0, 0)()
            k_chain(0, 0)()

            # just-in-time filler list for block (0, 0): (kb key, fn);
            # every V chain lands >=2 iterations ahead of its av(kb) reader
            jit00 = deque([
                (2, lambda: emit_v_chain(4)), (2, lambda: emit_v_chain(5)),
                (2, k_chain(0, 1)),
                (4, lambda: emit_v_chain(6)), (4, lambda: emit_v_chain(7)),
                (5, k_chain(0, 2)), (5, lambda: emit_v_chain(8)),
                (6, lambda: emit_v_chain(9)), (6, lambda: emit_v_chain(10)),
                (7, k_chain(0, 3)), (7, lambda: emit_v_chain(11)),
                (8, lambda: emit_v_chain(12)), (8, lambda: emit_v_chain(13)),
                (9, lambda: emit_v_chain(14)), (9, lambda: emit_v_chain(15)),
                (11, q_chain(0, 1)),
            ])

            filler_q = deque()
            pending = None
            for p in range(NPAIR):
                for qc in range(NQC):
                    # hazard guard: QK chains for pair p must be fully
                    # emitted before this pair's first scores read them
                    if qc == 0 and p > 0:
                        while filler_q:
                            filler_q.popleft()()
                    if p == 0 and qc == 1:
                        filler_q.append(q_chain(0, 2))
                    elif p == 0 and qc == 2:
                        build_qk(1)
                        filler_q.append(q_chain(0, 3))
                        for sc in range(4):
                            filler_q.append(k_chain(1, sc))
                        filler_q.append(q_chain(1, 0))
                    elif p == 0 and qc == 3:
                        for sc in range(1, 4):
                            filler_q.append(q_chain(1, sc))
                    elif p == 1 and qc == 2:
                        build_qk(2)
                        for sc in range(4):
                            filler_q.append(k_chain(2, sc))
                        filler_q.append(q_chain(2, 0))
                    elif p == 1 and qc == 3:
                        for sc in range(1, 4):
                            filler_q.append(q_chain(2, sc))
                    elif p == 2 and qc == 2:
                        build_qk(3)
                        for sc in range(4):
                            filler_q.append(k_chain(3, sc))
                        filler_q.append(q_chain(3, 0))
                    elif p == 2 and qc == 3:
                        for sc in range(1, 4):
                            filler_q.append(q_chain(3, sc))
                    elif p == 3 and qc >= 1:
                        filler_q.extend(cproj_fillers(qc - 1))

                    avs = av_ps.tile([128, QCW], f32, tag="avps",
                                     name=f"avps_{p}_{qc}")
                    accA = acc_pool.tile([128, 2, 2, 512], bf16, tag="accA",
                                         name=f"accA_{p}_{qc}")
                    accB = acc_pool.tile([128, 2, 2, 512], bf16, tag="accB",
                                         name=f"accB_{p}_{qc}")
                    pts = {}

                    def sc_exp(kb, p=p, qc=qc, pts=pts, accA=accA, accB=accB):
                        slot = kb % 2
                        st = st_ps.tile([128, 2, 512], f32, tag="st",
                                        name=f"st_{p}_{qc}_{kb}")
                        for h in range(2):
                            hsl = slice(64 * h, 64 * h + 64)
                            nc.tensor.matmul(
                                st[:, h, :],
                                lhsT=KT[p][hsl, kb * 128:(kb + 1) * 128],
                                rhs=QT[p][hsl, qc * QCW:(qc + 1) * QCW],
                                start=True, stop=True)
                        if slot == 0:
                            pt = pt_pool.tile([128, 2, 2, 512], bf16, tag="pt",
                                              name=f"pt_{p}_{qc}_{kb}")
                            pts[kb // 2] = pt
                        else:
                            pt = pts[kb // 2]
                        nc.scalar.activation(pt[:, :, slot, :], st[:], Exp,
                                             scale=SCALE)
                        if slot == 1:
                            pr = kb // 2
                            acc = accA if pr % 2 == 0 else accB
                            if pr < 2:
                                pass  # folded into the pr==2/3 first add
                            elif pr < 4:
                                nc.vector.tensor_add(acc[:], pts[pr - 2][:],
                                                     pt[:])
                            else:
                                nc.vector.tensor_add(acc[:], acc[:], pt[:])

                    def av(kb, p=p, avs=avs, pts=pts):
                        pt = pts[kb // 2]
                        for h in range(2):
                            nc.tensor.matmul(
                                avs[64 * h:64 * h + 64, :],
                                lhsT=VT[:, kb,
                                        p * 128 + 64 * h:p * 128 + 64 * h + 64],
                                rhs=pt[:, h, kb % 2, :],
                                start=(kb == 0), stop=(kb == KB - 1),
                                tile_position=(0, 64 * h),
                                skip_group_check=True)

                    sc_exp(0)
                    sc_exp(1)
                    if pending is not None:
                        emit_boundary(pending)
                    if p == 0 and qc == 0:
                        # V chains for the first key blocks go here, after
                        # the first scores, so they never head-block them
                        for vkb in range(4):
                            emit_v_chain(vkb)
                    av(0)
                    av(1)
                    for kb in range(2, KB):
                        sc_exp(kb)
                        if p == 0 and qc == 0:
                            while jit00 and jit00[0][0] <= kb:
                                jit00.popleft()[1]()
                        av(kb)
                        if (p, qc) != (0, 0) and kb % 2 == 0 and filler_q:
                            filler_q.popleft()()
                    pending = (p, qc, avs, accA, accB)

            emit_boundary(pending)
            while filler_q:
                filler_q.popleft()()
            # tail: alternate PSUM pools so chain->copy->chain never
            # serializes on a single buffer
            qc = NQC - 1
            for j, (sb, nn) in enumerate(
                    [(sb, nn) for sb in range(qc * 4, qc * 4 + 4)
                     for nn in range(2)]):
                emit_cproj_chain(sb, nn, pool=av_ps if j % 2 else None)

    nc.compile()
    return nc


def _get_nc():
    if "nc" not in _CACHED:
        _CACHED["nc"] = _build()
    return _CACHED["nc"]


def _shard(x, W_attn, W_proj):
    """Build per-core input maps (bf16)."""
    import ml_dtypes

    bf = ml_dtypes.bfloat16
    x = np.asarray(x, dtype=np.float32)
    W_attn = np.asarray(W_attn, dtype=np.float32)
    W_proj = np.asarray(W_proj, dtype=np.float32)
    in_maps = []
    for c in range(N_CORES):
        b, g = c // 2, c % 2
        fsl = slice(g * FC, (g + 1) * FC)
        in_maps.append({
            "xt": np.ascontiguousarray(x[b].T).astype(bf),
            "wq": np.ascontiguousarray(
                W_attn[:, 0 * D + g * FC:0 * D + (g + 1) * FC]).astype(bf),
            "wk": np.ascontiguousarray(
                W_attn[:, 1 * D + g * FC:1 * D + (g + 1) * FC]).astype(bf),
            "wv": np.ascontiguousarray(
                W_attn[:, 2 * D + g * FC:2 * D + (g + 1) * FC]).astype(bf),
            "wp": np.ascontiguousarray(
                W_proj[fsl, :].reshape(NPAIR, 128, D).transpose(1, 0, 2)
            ).astype(bf),
        })
    return in_maps


def kernel(x, W_attn, W_proj):
    from concourse.bass_utils import run_bass_kernel_spmd

    nc = _get_nc()
    in_maps = _shard(x, W_attn, W_proj)
    trace = os.environ.get("BASS_PROBLEM_TRACE", "0") == "1"
    res = run_bass_kernel_spmd(nc, in_maps, list(range(N_CORES)), trace=trace)
    _CACHED["last_result"] = res
    out = np.empty((B, S, D), dtype=np.float32)
    for b in range(B):
        out[b] = (np.asarray(res.results[2 * b]["out"], dtype=np.float32)
                  + np.asarray(res.results[2 * b + 1]["out"],
                               dtype=np.float32))
    return out
